# revision 10
# baseline (speedup 1.0000x reference)
"""Trainium2 Bass kernel for 3D multi-head attention (nn_Attention3D).

Problem: x [1, 16, 16, 16, 528] -> full attention over N=4096 tokens,
8 heads of dim 66, qkv + out projections.

Sharding: one head per NeuronCore (8 cores). Each core computes its
head's q/k/v projections, full 4096x4096 attention, and its partial
contribution to the output projection. Host sums the 8 partials and
adds the output bias.

Key layout decisions (all host-side prep, free):
  - x is pre-transposed on host to xT [640, 4096] (C on partitions),
    with row 528 = 1.0 (bias row) and rows 529-639 = 0 padding. This
    makes every on-device matmul contraction sit on the partition dim
    with K=128 chunks, with qkv biases folded into the weight matmuls.
  - q is pre-scaled by hd^-0.5 (folded into wq/bq on host).
  - v gets an extra ones-column (col 66), so the attention-value
    matmul also accumulates the softmax denominator for free.
  - Scores are computed transposed ([k-tokens, q-tokens]) so softmax's
    sum runs over the partition dim via the ones-column trick, exp runs
    on ScalarE straight out of PSUM, and no transposes are ever needed.
  - Matmul operands are float32r (fast fp32 mode, 1 cycle/row at
    N>=512, ~2^-13 rel error, fp32 accumulation in PSUM). fp16 was
    measured at 2 cycles/row on hardware; bf16 is 1 cycle/row but
    ~2^-8. float32r needs f32r-typed producers and even innermost AP
    sizes everywhere (hence the 68-wide v tile).
"""

import numpy as np

EMBED = 528
HD = 66
NHEADS = 8
NT = 4096
NCH = 5  # contraction chunks of 128 (640 = 528 + bias row + pad)
G = 3  # k-chunks per exp group (3 PSUM banks per scores tile)


def _build_nc(nt=NT):
    import concourse.tile as tile
    from concourse import bacc, mybir

    F32 = mybir.dt.float32
    F32R = mybir.dt.float32r  # fast fp32 matmul mode: 1 cyc/row at N>=256
    AF = mybir.ActivationFunctionType

    nkc = nt // 128  # k-token chunks
    nqb = nt // 512  # q-token blocks
    ntb = nt // 128  # token blocks for the projection

    nc = bacc.Bacc("TRN2", target_bir_lowering=False, debug=False)
    xT_d = nc.dram_tensor("xT", [NCH, 128, nt], F32R, kind="ExternalInput").ap()
    wq_d = nc.dram_tensor("wq", [128, NCH, 128], F32R, kind="ExternalInput").ap()
    wk_d = nc.dram_tensor("wk", [128, NCH, 128], F32R, kind="ExternalInput").ap()
    z_d = nc.dram_tensor("zeros", [128, nt], F32R, kind="ExternalInput").ap()
    wv_d = nc.dram_tensor("wv", [128, NCH, HD + 2], F32R, kind="ExternalInput").ap()
    wp_d = nc.dram_tensor("wp", [128, EMBED], F32R, kind="ExternalInput").ap()
    y_d = nc.dram_tensor("y", [nt, EMBED], F32, kind="ExternalOutput").ap()

    with tile.TileContext(nc) as tc:
        with (
            tc.tile_pool(name="const", bufs=1) as constp,
            tc.tile_pool(name="persist", bufs=1) as pp,
        ):
            wq = constp.tile([128, NCH, 128], F32R, name="wq_sb")
            wk = constp.tile([128, NCH, 128], F32R, name="wk_sb")
            wv = constp.tile([128, NCH, HD + 2], F32R, name="wv_sb")
            wp = constp.tile([128, EMBED], F32R, name="wp_sb")
            nc.sync.dma_start(wq[:], wq_d[:])
            nc.sync.dma_start(wk[:], wk_d[:])
            nc.sync.dma_start(wv[:], wv_d[:])
            nc.sync.dma_start(wp[:], wp_d[:])

            # qT/kT/oT are hd-padded to 128 partitions (rows HD.. stay 0) so
            # every matmul contracts over a full K=128.
            qT = pp.tile([128, nt], F32R, name="qT")
            kT = pp.tile([128, nt], F32R, name="kT")
            oT = pp.tile([128, nt], F32R, name="oT")
            vaug = pp.tile([128, nkc, HD + 2], F32R, name="vaug")
            recipT = pp.tile([128, ntb], F32, name="recipT")
            # oT rows 67-127 must be zero for the projection matmul; qT/kT
            # rows 66-127 are computed zeros (host-padded wq/wk columns).
            nc.sync.dma_start(oT[:], z_d[:])

            # ---------------- Phase A: qkv projections ----------------
            with (
                tc.tile_pool(name="xp", bufs=1) as xp,
                tc.tile_pool(name="psA", bufs=2, space="PSUM") as psA,
            ):
                xT = xp.tile([128, NCH, nt], F32R, name="xT_sb")
                # chunked DMA so compute can start before the full 10MB lands
                for b in range(nqb):
                    qs = slice(b * 512, (b + 1) * 512)
                    for c in range(NCH):
                        nc.sync.dma_start(xT[:, c, qs], xT_d[c, :, qs])

                for b in range(nqb):
                    qs = slice(b * 512, (b + 1) * 512)
                    for w, dst in ((wq, qT), (wk, kT)):
                        ps = psA.tile([128, 512], F32, tag="qk", name="ps_qk")
                        for c in range(NCH):
                            nc.tensor.matmul(
                                ps[:],
                                w[:, c, :],
                                xT[:, c, qs],
                                start=(c == 0),
                                stop=(c == NCH - 1),
                            )
                        nc.vector.tensor_copy(dst[:, qs], ps[:])
                for t in range(nkc):
                    ts_ = slice(t * 128, (t + 1) * 128)
                    psv = psA.tile([128, HD + 2], F32, tag="v", name="ps_v")
                    for c in range(NCH):
                        nc.tensor.matmul(
                            psv[:],
                            xT[:, c, ts_],
                            wv[:, c, :],
                            start=(c == 0),
                            stop=(c == NCH - 1),
                        )
                    nc.vector.tensor_copy(vaug[:, t, :], psv[:])

            # ---------------- Phase B: attention ----------------
            groups = []
            kc0 = 0
            while kc0 < nkc:
                groups.append((kc0, min(G, nkc - kc0)))
                kc0 += G

            with (
                tc.tile_pool(name="ep", bufs=3) as ep,
                tc.tile_pool(name="rp", bufs=2) as rp,
                tc.tile_pool(name="drp", bufs=2, space="DRAM") as drp,
                tc.tile_pool(name="psS", bufs=2, space="PSUM") as psS,
                tc.tile_pool(name="psO", bufs=2, space="PSUM") as psO,
            ):
                for b in range(nqb):
                    qs = slice(b * 512, (b + 1) * 512)
                    o_ps = psO.tile([HD + 2, 512], F32, name="o_ps")
                    for g0, gsz in groups:
                        sc = psS.tile([128, G * 512], F32, name="sc")
                        for j in range(gsz):
                            kc = g0 + j
                            nc.tensor.matmul(
                                sc[:, j * 512 : (j + 1) * 512],
                                kT[:, kc * 128 : (kc + 1) * 128],
                                qT[:, qs],
                                start=True,
                                stop=True,
                            )
                        E = ep.tile([128, G * 512], F32R, name="E")
                        nc.scalar.activation(
                            E[:, : gsz * 512], sc[:, : gsz * 512], AF.Exp
                        )
                        for j in range(gsz):
                            kc = g0 + j
                            nc.tensor.matmul(
                                o_ps[:],
                                vaug[:, kc, :],
                                E[:, j * 512 : (j + 1) * 512],
                                start=(kc == 0),
                                stop=(kc == nkc - 1),
                                skip_group_check=True,
                            )
                    recip = rp.tile([1, 512], F32, name="recip")
                    nc.vector.reciprocal_approx_fast(recip[:], o_ps[0:1, :])
                    dstage = drp.tile([1, 512], F32, name="dstage")
                    nc.sync.dma_start(dstage[:], recip[:])
                    nc.sync.dma_start(
                        recipT[:, b * 4 : (b + 1) * 4],
                        dstage.rearrange("o (f p) -> (o p) f", p=128),
                    )
                    nc.vector.tensor_copy(oT[: HD + 2, qs], o_ps[:])

            # ---------------- Phase C: output projection ----------------
            with (
                tc.tile_pool(name="yp", bufs=3) as yp,
                tc.tile_pool(name="psY", bufs=2, space="PSUM") as psY,
            ):
                for t in range(ntb):
                    ts_ = slice(t * 128, (t + 1) * 128)
                    yps = psY.tile([128, 1024], F32, name="yps")
                    nc.tensor.matmul(
                        yps[:, :512],
                        oT[:, ts_],
                        wp[:, :512],
                        start=True,
                        stop=True,
                    )
                    nc.tensor.matmul(
                        yps[:, 512 : 512 + (EMBED - 512)],
                        oT[:, ts_],
                        wp[:, 512:],
                        start=True,
                        stop=True,
                    )
                    ysb = yp.tile([128, EMBED], F32, name="ysb")
                    nc.vector.tensor_scalar_mul(
                        ysb[:], yps[:, :EMBED], recipT[:, t : t + 1]
                    )
                    nc.sync.dma_start(y_d[ts_, :], ysb[:])

    nc.compile()
    return nc


def _prep_inputs(x, w_qkv, b_qkv, w_proj, nt):
    """Host-side shard prep: returns list of 8 in_maps."""
    x = np.asarray(x, dtype=np.float32)
    w_qkv = np.asarray(w_qkv, dtype=np.float32)
    b_qkv = np.asarray(b_qkv, dtype=np.float32)
    w_proj = np.asarray(w_proj, dtype=np.float32)

    xt = x.reshape(nt, EMBED)
    xT_pad = np.zeros((NCH * 128, nt), dtype=np.float32)
    xT_pad[:EMBED] = xt.T
    xT_pad[EMBED] = 1.0
    xT_in = np.ascontiguousarray(xT_pad.reshape(NCH, 128, nt))

    s = float(HD) ** -0.5
    in_maps = []
    for h in range(NHEADS):
        sl_q = slice(h * HD, (h + 1) * HD)
        sl_k = slice(EMBED + h * HD, EMBED + (h + 1) * HD)
        sl_v = slice(2 * EMBED + h * HD, 2 * EMBED + (h + 1) * HD)

        wq_t = np.zeros((NCH * 128, 128), dtype=np.float32)
        wq_t[:EMBED, :HD] = (w_qkv[sl_q] * s).T
        wq_t[EMBED, :HD] = b_qkv[sl_q] * s

        wk_t = np.zeros((NCH * 128, 128), dtype=np.float32)
        wk_t[:EMBED, :HD] = w_qkv[sl_k].T
        wk_t[EMBED, :HD] = b_qkv[sl_k]

        # ones column sits at index 0 so the softmax denominator lands on
        # PSUM partition 0 (engine partition bases must be 32-aligned)
        # fp32r matmuls need even innermost sizes -> pad to 68 columns
        wv_t = np.zeros((NCH * 128, HD + 2), dtype=np.float32)
        wv_t[:EMBED, 1 : HD + 1] = w_qkv[sl_v].T
        wv_t[EMBED, 1 : HD + 1] = b_qkv[sl_v]
        wv_t[EMBED, 0] = 1.0  # ones column -> softmax denominator

        wp_t = np.zeros((128, EMBED), dtype=np.float32)
        wp_t[1 : HD + 1] = w_proj[:, sl_q].T  # row 0 = 0 kills the denom row

        in_maps.append(
            {
                "xT": xT_in,
                "wq": np.ascontiguousarray(
                    wq_t.reshape(NCH, 128, 128).transpose(1, 0, 2)
                ),
                "wk": np.ascontiguousarray(
                    wk_t.reshape(NCH, 128, 128).transpose(1, 0, 2)
                ),
                "zeros": np.zeros((128, nt), dtype=np.float32),
                "wv": np.ascontiguousarray(
                    wv_t.reshape(NCH, 128, HD + 2).transpose(1, 0, 2)
                ),
                "wp": wp_t,
            }
        )
    return in_maps


_NC_CACHE = {}


def _get_nc(nt=NT):
    if nt not in _NC_CACHE:
        _NC_CACHE[nt] = _build_nc(nt)
    return _NC_CACHE[nt]


def kernel(x, w_qkv, b_qkv, w_proj, b_proj, _trace=False):
    from concourse.bass_utils import run_bass_kernel_spmd

    x = np.asarray(x, dtype=np.float32)
    b_proj = np.asarray(b_proj, dtype=np.float32)
    B, D, H, W, C = x.shape
    nt = D * H * W

    nc = _get_nc(nt)
    in_maps = _prep_inputs(x, w_qkv, b_qkv, w_proj, nt)
    res = run_bass_kernel_spmd(
        nc, in_maps, core_ids=list(range(NHEADS)), trace=_trace
    )
    out = np.zeros((nt, EMBED), dtype=np.float32)
    for r in res.results:
        out += r["y"]
    out += b_proj
    kernel.last_results = res
    return out.reshape(B, D, H, W, C)
